# revision 51
# baseline (speedup 1.0000x reference)
"""CardAwarePolicy Trainium2 kernel (8-core data parallel).

Key idea: q/k/v of a hand card depend only on its card id (embedding row ->
linear proj).  So attention collapses into card-id space:
  - per-card tables Qtab/Ktab/Vtab [54, 64] (bias folded in)
  - per-head exp score tables E_h[ci, cj] = exp(q_h(ci) . k_h(cj) / 4)  [54x54]
  - per-sample histogram h[c] = #positions with card c (incl. c=0),
    n[c] = h[c] for c>0 (masked keys), hsp[c] = h[c] / max(hand_size,1)
  - den[b,h,ci] = sum_{cj>0} E_h[ci,cj] n[cj]            (matmul)
  - W[b,h,ci]  = hsp[ci] / den[b,h,ci]                   (divide)
  - S[b,h,cj]  = sum_ci E_h[ci,cj] W[b,h,ci]             (matmul)
  - U[b,h,cj]  = h[cj] * S[b,h,cj]  (cj=0 rows killed by zero rows in BDV)
  - handsum[b,(h,d)] = sum_cj Vtab[cj,(h,d)] U[b,(h,cj)] (block-diag matmul)
    + augmented 65th row = 8/len(b) which carries out_b through the
    fused (c1_w @ out_w) projection.
Then plain batched MLPs in feature-major [feat, batch] layout.

Structure notes (HW-tuned):
  - inputs are host-packed to contiguous per-partition rows and loaded to
    SBUF ONCE; the rep loop (`For_i(0, reps)`) re-runs the whole per-core
    batch with compile-time addresses and stores the output once per rep,
    so the NEFF size is independent of `reps` (clean rep-delta timing).
  - NO GPSIMD compute in the loop: each Pool op costs ~10us dispatch on
    real TRN2 (ucode), which CoreSim's cost model does not reflect.
  - fp32r everywhere on the matmul paths (1 cyc/row vs 4 for fp32);
    bf16 on the h1/h2 action path (DVE 2x + bf16 matmuls).
  - histogram: one bf16 broadcast-compare + bf16 tree-add over positions,
    then one 128x128 transpose per tile yields [hspT || hT] together.
"""
import sys

if "/opt/trn_rl_repo" not in sys.path:
    sys.path.insert(0, "/opt/trn_rl_repo")

import numpy as np
from contextlib import ExitStack

import concourse.bass as bass
import concourse.tile as tile
from concourse import mybir
from concourse.masks import make_identity

F32 = mybir.dt.float32
BF = mybir.dt.bfloat16
I32 = mybir.dt.int32
OP = mybir.AluOpType
AF = mybir.ActivationFunctionType

B, S, A, KC = 32768, 8, 20, 4
E, H, D = 64, 4, 16
NCARD = 54
NCORES = 8
BC = B // NCORES  # 4096 per core


def legalize_multiwait(nc):
    """Split >1 sem waits on Drain/CTRL instructions (walrus limit) into
    preceding single-wait EventSemaphore carriers.  Also move non-unit
    sem updates (sem-add-imm N) off compute instructions onto trailing
    EventSemaphore carriers — only EventSemaphore encodes those."""
    for fn in nc.m.functions:
        for blk in fn.blocks:
            new_list = []
            for inst in blk.instructions:
                si = inst.sync_info
                if si and si.on_wait and len(si.on_wait) > 1:
                    waits = list(si.on_wait)
                    for w in waits[:-1]:
                        nm = f"{inst.name}-wsplit-{w.id}"
                        d = mybir.InstEventSemaphore(name=nm, ins=[], outs=[])
                        d.engine = inst.engine
                        d.sync_info = mybir.SyncInfo(on_wait=[w], on_update=[])
                        nc.register_instruction(d, overwrite=True)
                        new_list.append(d)
                    si.on_wait[:] = [waits[-1]]
                new_list.append(inst)
                if (si and si.on_update
                        and not isinstance(inst, (mybir.InstEventSemaphore,
                                                  mybir.InstNoOp,
                                                  mybir.InstDrain))):
                    big = [u for u in si.on_update
                           if getattr(u, "update_value", 1) not in (None, 1, 16)]
                    if big:
                        keep = [u for u in si.on_update if u not in big]
                        nm = f"{inst.name}-usplit"
                        # Drain waits for the engine's in-flight work to
                        # COMPLETE before its updates fire — safe carrier.
                        d = mybir.InstDrain(name=nm, ins=[], outs=[])
                        d.engine = inst.engine
                        d.sync_info = mybir.SyncInfo(on_wait=[], on_update=big)
                        nc.register_instruction(d, overwrite=True)
                        si.on_update[:] = keep
                        new_list.append(d)
            blk.instructions[:] = new_list
    return nc


def build_nc(b_core=BC, nva=A, gps_hist=0, gps_h1=0, act_h1=2, act_h2=5, reps=1,
             stage=9, UN=2, pool_off=True, hints=True, act_dup=True,
             dve_copies=True, SBB=3, SBH=4):
    if pool_off:
        gps_h1 = 0  # GPSIMD ops cost ~10us dispatch on real HW: keep it idle
    gps_hist = 0  # Pool rejects the stride-0 broadcast compare (ISA check)
    CH = 512
    assert b_core % CH == 0
    nchunks = b_core // CH
    if nchunks % UN != 0:
        UN = 1
    NT = CH // 128  # 4 batch tiles per chunk
    NC2 = 64        # padded card space (cards 54..63 are phantoms: h[c]=0)

    TB = b_core // 128
    nc = bass.Bass()
    dp = nc.declare_dram_parameter
    cards_d = dp("hand_cards", [128, TB, S], I32, isOutput=False)
    gs_d = dp("game_state", [12, b_core], F32, isOutput=False)
    hsz_d = dp("hand_size", [128, TB], I32, isOutput=False)
    aci_d = dp("action_card_indices", [A, KC], I32, isOutput=False)
    acc_d = dp("action_card_counts", [A], I32, isOutput=False)
    emb_d = dp("emb", [NCARD, E], F32, isOutput=False)
    ipw_d = dp("in_proj_w", [3 * E, E], F32, isOutput=False)
    ipb_d = dp("in_proj_b", [3 * E], F32, isOutput=False)
    ow_d = dp("out_w", [E, E], F32, isOutput=False)
    ob_d = dp("out_b", [E], F32, isOutput=False)
    g1w_d = dp("g1_w", [64, 12], F32, isOutput=False)
    g1b_d = dp("g1_b", [64], F32, isOutput=False)
    g2w_d = dp("g2_w", [32, 64], F32, isOutput=False)
    g2b_d = dp("g2_b", [32], F32, isOutput=False)
    c1w_d = dp("c1_w", [128, 96], F32, isOutput=False)
    c1b_d = dp("c1_b", [128], F32, isOutput=False)
    c2w_d = dp("c2_w", [128, 128], F32, isOutput=False)
    c2b_d = dp("c2_b", [128], F32, isOutput=False)
    s1w_d = dp("s1_w", [64, 192], F32, isOutput=False)
    s1b_d = dp("s1_b", [64], F32, isOutput=False)
    s2w_d = dp("s2_w", [32, 64], F32, isOutput=False)
    s2b_d = dp("s2_b", [32], F32, isOutput=False)
    s3w_d = dp("s3_w", [1, 32], F32, isOutput=False)
    s3b_d = dp("s3_b", [1], F32, isOutput=False)
    out_d = dp("out", [128, TB, A], F32, isOutput=True)

    R = mybir.dt.float32r

    def bcast_ap(dram_ap, parts):
        return bass.AP(tensor=dram_ap.tensor, offset=dram_ap.offset,
                       ap=[[0, parts]] + list(dram_ap.ap))

    with tile.TileContext(nc) as tc:
        with ExitStack() as ctx:
            const = ctx.enter_context(tc.tile_pool(name="const", bufs=1))
            ps0_cm = tc.tile_pool(name="ps0", bufs=4, space="PSUM")
            ps0 = ps0_cm.__enter__()

            dma = nc.sync.dma_start
            _dmae = [nc.sync, nc.gpsimd, nc.scalar]
            _dmac = [0]

            def cdma(**kw):
                e = _dmae[_dmac[0] % len(_dmae)]
                _dmac[0] += 1
                e.dma_start(**kw)

            # ---------------- phase 0: constants & tables ----------------
            ident = const.tile([128, 128], F32)
            make_identity(nc, ident)

            def pe_T(in_ap, out_shape, name):
                p = ps0.tile(out_shape, F32, name=name, tag="p0")
                kdim = in_ap.shape[0]
                nc.tensor.matmul(p, in_ap, ident[0:kdim, 0:kdim], is_transpose=True)
                return p

            def evac(name, psum_ap, bias=None, dt=F32):
                t = const.tile(list(psum_ap.shape), dt, tag=name, name=name)
                if bias is None:
                    nc.vector.tensor_copy(t, psum_ap)
                else:
                    nc.vector.tensor_scalar_add(t, psum_ap, bias)
                return t

            # raw small loads
            embS = const.tile([NCARD, E], F32)
            cdma(out=embS, in_=emb_d[:, :])
            ipw_lo = const.tile([128, E], F32)
            cdma(out=ipw_lo, in_=ipw_d[0:128, :])
            ipw_hi = const.tile([64, E], F32)
            cdma(out=ipw_hi, in_=ipw_d[128:192, :])
            owS = const.tile([E, E], F32)
            cdma(out=owS, in_=ow_d[:, :])
            g1wS = const.tile([64, 12], F32)
            cdma(out=g1wS, in_=g1w_d[:, :])
            g2wS = const.tile([32, 64], F32)
            cdma(out=g2wS, in_=g2w_d[:, :])
            c1wS = const.tile([128, 96], F32)
            cdma(out=c1wS, in_=c1w_d[:, :])
            c2wS = const.tile([128, 128], F32)
            cdma(out=c2wS, in_=c2w_d[:, :])
            s1wS = const.tile([64, 192], F32)
            cdma(out=s1wS, in_=s1w_d[:, :])
            s2wS = const.tile([32, 64], F32)
            cdma(out=s2wS, in_=s2w_d[:, :])
            aciS = const.tile([A, KC], I32)
            cdma(out=aciS, in_=aci_d[:, :])
            accS = const.tile([A, 1], I32)
            cdma(out=accS, in_=acc_d[:].unsqueeze(1))

            # bias columns
            qb = const.tile([E, 1], F32)
            cdma(out=qb, in_=ipb_d[0:64].unsqueeze(1))
            kb = const.tile([E, 1], F32)
            cdma(out=kb, in_=ipb_d[64:128].unsqueeze(1))
            vb = const.tile([E, 1], F32)
            cdma(out=vb, in_=ipb_d[128:192].unsqueeze(1))
            g1b = const.tile([64, 1], F32)
            cdma(out=g1b, in_=g1b_d[:].unsqueeze(1))
            g2b = const.tile([32, 1], F32)
            cdma(out=g2b, in_=g2b_d[:].unsqueeze(1))
            c1b = const.tile([128, 1], F32)
            cdma(out=c1b, in_=c1b_d[:].unsqueeze(1))
            c2b = const.tile([128, 1], F32)
            cdma(out=c2b, in_=c2b_d[:].unsqueeze(1))
            s1b = const.tile([64, 1], F32)
            cdma(out=s1b, in_=s1b_d[:].unsqueeze(1))
            s2b4 = const.tile([128, 1], F32)
            for r in range(4):
                cdma(out=s2b4[r * 32:(r + 1) * 32, :], in_=s2b_d[:].unsqueeze(1))
            w3col = const.tile([32, 1], F32)
            cdma(out=w3col, in_=s3w_d[0, :].unsqueeze(1))
            s3b_bc = const.tile([128, 1], F32)
            cdma(out=s3b_bc, in_=bcast_ap(s3b_d[0:1].unsqueeze(1), 128))

            # transposed weights
            embT = const.tile([E, NC2], R)      # [64(e), 64(c)] phantom cols = 0
            nc.vector.memset(embT.bitcast(F32), 0.0)
            embT_p = pe_T(embS, [E, NCARD], "embT_p")
            nc.vector.tensor_copy(embT[:, 0:NCARD], embT_p)
            wT_lo = pe_T(ipw_lo, [E, 128], "wT_lo")
            wT_hi = pe_T(ipw_hi, [E, 64], "wT_hi")
            WT = const.tile([E, 3 * E], R)
            nc.vector.tensor_copy(WT[:, 0:128], wT_lo)
            nc.vector.tensor_copy(WT[:, 128:192], wT_hi)
            g1wT = evac("g1wT", pe_T(g1wS, [12, 64], "g1wT_p"))
            g2wT = evac("g2wT", pe_T(g2wS, [64, 32], "g2wT_p"), dt=R)
            c1wT = evac("c1wT", pe_T(c1wS, [96, 128], "c1wT_p"), dt=R)
            c2wT = evac("c2wT", pe_T(c2wS, [128, 128], "c2wT_p"), dt=R)
            s1wT_A = evac("s1wT_A", pe_T(s1wS[:, 0:128], [128, 64], "s1wT_A_p"), dt=R)
            s1wT_B = evac("s1wT_B", pe_T(s1wS[:, 128:192], [64, 64], "s1wT_B_p"))
            s2wT = evac("s2wT", pe_T(s2wS, [64, 32], "s2wT_p"), dt=R)
            c1wB = const.tile([32, 128], R)
            cdma(out=c1wB, in_=c1wT[64:96, :])
            s1wAd = const.tile([128, 128], R)
            cdma(out=s1wAd[:, 0:64], in_=s1wT_A[:, :])
            cdma(out=s1wAd[:, 64:128], in_=s1wT_A[:, :])

            # q/k/v per-card tables [64(f), 64(c)] with bias folded
            def tab(which, bias, name, dt=F32):
                p = ps0.tile([E, NC2], F32, name=name + "_p", tag="p0")
                nc.tensor.matmul(p, WT[:, which * E:(which + 1) * E], embT, start=True, stop=True)
                return evac(name, p, bias, dt=dt)

            QtabT = tab(0, qb, "QtabT", dt=R)
            KtabT = tab(1, kb, "KtabT", dt=R)
            VtabT = tab(2, vb, "VtabT")

            QH = [const.tile([D, NC2], R, name=f"QH{h}", tag=f"QH{h}") for h in range(H)]
            KH = [const.tile([D, NC2], R, name=f"KH{h}", tag=f"KH{h}") for h in range(H)]
            for h in range(H):
                cdma(out=QH[h], in_=QtabT[h * D:(h + 1) * D, :])
                cdma(out=KH[h], in_=KtabT[h * D:(h + 1) * D, :])
            sc_p = ps0.tile([NC2, H * NC2], F32, tag="p0")
            sc2_p = ps0.tile([NC2, H * NC2], F32, tag="p0")
            for h in range(H):
                nc.tensor.matmul(sc_p[:, h * NC2:(h + 1) * NC2], QH[h], KH[h], start=True, stop=True)
                nc.tensor.matmul(sc2_p[:, h * NC2:(h + 1) * NC2], KH[h], QH[h], start=True, stop=True)
            EtabF0 = const.tile([NC2, H * NC2], F32)  # E[ci, h*64+cj]
            E2tab = const.tile([NC2, H * NC2], R)     # E[cj, h*64+ci]
            nc.scalar.activation(EtabF0, sc_p, AF.Exp, scale=1.0 / float(np.sqrt(D)))
            nc.scalar.activation(E2tab.bitcast(F32), sc2_p, AF.Exp, scale=1.0 / float(np.sqrt(D)))
            EtabF = const.tile([NC2, H * NC2], R)
            nc.vector.tensor_copy(EtabF, EtabF0)
            E2tabr = const.tile([NC2, H * NC2], R)
            nc.vector.tensor_copy(E2tabr, E2tab.bitcast(F32))
            # cj=0 keys are padding: kill them inside den's table
            nc.vector.memset(E2tabr.bitcast(F32)[0:1, :], 0.0)
            # block-diag full-E pairs for the S matmul: [128(h,ci), 128(h,cj)]
            BDEf = []
            for g in range(2):
                bde = const.tile([128, 128], R, name=f"BDEf{g}", tag=f"BDEf{g}")
                nc.vector.memset(bde.bitcast(F32), 0.0)
                cdma(out=bde[0:64, 0:64],
                     in_=EtabF[:, (2 * g) * NC2:(2 * g + 1) * NC2])
                cdma(out=bde[64:128, 64:128],
                     in_=EtabF[:, (2 * g + 1) * NC2:(2 * g + 2) * NC2])
                BDEf.append(bde)

            # Vtab (card-major) [64, 64]
            vt_p = ps0.tile([NC2, E], F32, tag="p0")
            nc.tensor.matmul(vt_p, VtabT.bitcast(F32), ident[0:E, 0:E], is_transpose=True)
            Vtab = evac("Vtab", vt_p, dt=R)
            # BDV pairs [128, 65]: rows 0..63 head 2g, 64..127 head 2g+1; aug col
            # on h0 rows.  cj=0 rows (0 and 64) stay zero: U[0] never computed
            # clean now that n keeps its bin-0 count.
            BDVP = []
            for g in range(2):
                b = const.tile([128, E + 1], R, name=f"BDVP{g}", tag=f"BDVP{g}")
                nc.vector.memset(b.bitcast(F32), 0.0)
                cdma(out=b[0:64, (2 * g) * D:(2 * g + 1) * D],
                    in_=Vtab[:, (2 * g) * D:(2 * g + 1) * D])
                cdma(out=b[64:128, (2 * g + 1) * D:(2 * g + 2) * D],
                    in_=Vtab[:, (2 * g + 1) * D:(2 * g + 2) * D])
                nc.vector.memset(b.bitcast(F32)[0:1, (2 * g) * D:(2 * g + 1) * D], 0.0)
                nc.vector.memset(b.bitcast(F32)[64:65, (2 * g + 1) * D:(2 * g + 2) * D], 0.0)
                BDVP.append(b)
            nc.vector.memset(BDVP[0].bitcast(F32)[0:64, E:E + 1], 1.0)
            nc.vector.memset(BDVP[0].bitcast(F32)[0:1, E:E + 1], 0.0)

            # fused (c1_w[:, :64] @ out_w_aug) projection  M1T [65, 128]
            ob_col = const.tile([E, 1], F32)
            cdma(out=ob_col, in_=ob_d[:].unsqueeze(1))
            ow_aug2 = const.tile([E, E + 1], R)
            nc.vector.tensor_copy(ow_aug2[:, 0:E], owS)
            nc.vector.tensor_copy(ow_aug2[:, E:E + 1], ob_col)
            m1_p = ps0.tile([E + 1, 128], F32, tag="p0")
            nc.tensor.matmul(m1_p, ow_aug2, c1wT[0:64, :], start=True, stop=True)
            M1T = evac("M1T", m1_p, dt=R)
            # [handsum; g2] -> c1 as ONE matmul: m1cat = [M1T ; c1wB] [97, 128]
            m1cat = const.tile([128, 128], R)
            nc.vector.memset(m1cat.bitcast(F32), 0.0)
            nc.vector.tensor_copy(m1cat[0:E + 1, :], M1T)
            nc.vector.tensor_copy(m1cat[96:128, :], c1wT[64:96, :])

            # BD_s2w [128, 64] bf16, BD4_w3 [128, 4] bf16
            BDs2L = const.tile([128, 128], BF)
            nc.vector.memset(BDs2L.bitcast(F32), 0.0)
            nc.vector.tensor_copy(BDs2L[0:64, 0:32], s2wT.bitcast(F32))
            nc.vector.tensor_copy(BDs2L[64:128, 32:64], s2wT.bitcast(F32))
            BDs2R = const.tile([128, 128], BF)
            nc.vector.memset(BDs2R.bitcast(F32), 0.0)
            nc.vector.tensor_copy(BDs2R[0:64, 64:96], s2wT.bitcast(F32))
            nc.vector.tensor_copy(BDs2R[64:128, 96:128], s2wT.bitcast(F32))
            BDw3 = const.tile([128, 4], BF)
            nc.vector.memset(BDw3.bitcast(F32), 0.0)
            for a in range(4):
                nc.vector.tensor_copy(BDw3[a * 32:(a + 1) * 32, a:a + 1], w3col)

            # iota constants
            iota_i = const.tile([128, NC2], I32)
            nc.gpsimd.iota(iota_i, pattern=[[1, NC2]], base=0, channel_multiplier=0)
            iota_f = const.tile([128, NC2], F32)
            nc.vector.tensor_copy(iota_f, iota_i)
            iota_bf = const.tile([128, NC2], BF)
            nc.vector.tensor_copy(iota_bf, iota_i)
            iota_a = const.tile([A, NCARD], F32)
            nc.vector.tensor_copy(iota_a, iota_f[0:A, 0:NCARD])

            # ---- action table -> Ptab [64, 20] -> Ptab2 [128, 10] ----
            accf = const.tile([A, 1], F32)
            nc.vector.tensor_copy(accf, accS)
            acif = const.tile([A, KC], F32)
            nc.vector.tensor_copy(acif, aciS)
            wact = const.tile([A, NCARD], F32)
            tmp_e = const.tile([A, NCARD], F32)
            for k in range(KC):
                mk = const.tile([A, 1], F32, name=f"mk{k}", tag=f"mk{k}")
                nc.vector.tensor_scalar(mk, accf, float(k), None, OP.is_gt)
                mk_b = bass.AP(tensor=mk.tensor, offset=mk.offset,
                               ap=[mk.ap[0], [0, NCARD]])
                if k == 0:
                    nc.vector.scalar_tensor_tensor(
                        out=wact, in0=iota_a, scalar=acif[:, 0:1], in1=mk_b,
                        op0=OP.is_equal, op1=OP.mult)
                else:
                    nc.vector.scalar_tensor_tensor(
                        out=tmp_e, in0=iota_a, scalar=acif[:, k:k + 1], in1=mk_b,
                        op0=OP.is_equal, op1=OP.mult)
                    nc.vector.tensor_add(wact, wact, tmp_e)
            cmax = const.tile([A, 1], F32)
            nc.vector.tensor_scalar_max(cmax, accf, 1.0)
            crec = const.tile([A, 1], F32)
            nc.vector.reciprocal(crec, cmax)
            nc.vector.tensor_scalar_mul(wact, wact, crec)
            wac_p = ps0.tile([NCARD, A], F32, tag="p0")
            nc.tensor.matmul(wac_p, wact.bitcast(F32), ident[0:A, 0:A], is_transpose=True)
            WAC = evac("WAC", wac_p)
            arep_p = ps0.tile([E, A], F32, tag="p0")
            nc.tensor.matmul(arep_p, embS, WAC, start=True, stop=True)
            arepT = evac("arepT", arep_p)
            pt_p = ps0.tile([64, A], F32, tag="p0")
            nc.tensor.matmul(pt_p, s1wT_B, arepT, start=True, stop=True)
            Ptab = evac("Ptab", pt_p, s1b)
            Ptab2 = const.tile([128, A // 2], F32)
            cdma(out=Ptab2[0:64, :], in_=Ptab.rearrange("f (j two) -> f j two", two=2)[:, :, 0])
            cdma(out=Ptab2[64:128, :], in_=Ptab.rearrange("f (j two) -> f j two", two=2)[:, :, 1])

            ps0_cm.__exit__(None, None, None)

            # ---------------- per-chunk pipeline ----------------
            sb = ctx.enter_context(tc.tile_pool(name="sb", bufs=SBB))
            sbh = ctx.enter_context(tc.tile_pool(name="sbh", bufs=SBH))
            sbh2 = ctx.enter_context(tc.tile_pool(name="sbh2", bufs=10))
            psA = ctx.enter_context(tc.tile_pool(name="psA", bufs=2, space="PSUM"))
            psB = ctx.enter_context(tc.tile_pool(name="psB", bufs=2, space="PSUM"))
            psC = ctx.enter_context(tc.tile_pool(name="psC", bufs=2, space="PSUM"))
            psD = ctx.enter_context(tc.tile_pool(name="psD", bufs=2, space="PSUM"))

            # whole-core inputs resident in SBUF: ONE contiguous load each
            cardsAll = const.tile([128, TB, S], I32)
            cdma(out=cardsAll, in_=cards_d[:, :, :])
            gsAll = const.tile([12, b_core], F32)
            cdma(out=gsAll, in_=gs_d[:, :])
            hszAll = const.tile([128, TB], I32)
            cdma(out=hszAll, in_=hsz_d[:, :])
            c4bAll = const.tile([128, TB, S], BF)
            hsmAll = const.tile([128, TB], F32)
            rlAll = const.tile([128, TB], F32)
            out_sb = const.tile([128, TB, A], F32)

            def rep_prologue():
                nc.vector.tensor_copy(c4bAll, cardsAll)
                nc.vector.tensor_scalar_max(hsmAll, hszAll, 1.0)
                nc.vector.reciprocal(rlAll, hsmAll)

            def body(cix):
                gsT = gsAll[:, cix * CH:(cix + 1) * CH]
                rl = rlAll[:, cix * NT:(cix + 1) * NT]

                # one-shot histogram: cmp[p,t,c,s] = (cards[p,t,s] == c) bf16,
                # tree-add over s (bf16 pairs run 2x on DVE); TI[:, t] holds
                # [hsp || h] so ONE 128x128 transpose/tile yields both halves.
                cmp = sbh.tile([128, NT, NC2, S], BF, tag="cmp")
                TI = sb.tile([128, NT, 128], F32, tag="TI")
                for (t0, t1, eng) in ((0, gps_hist, nc.gpsimd),
                                      (gps_hist, NT, nc.vector)):
                    if t0 >= t1:
                        continue
                    c4s = c4bAll[:, cix * NT + t0:cix * NT + t1, :]
                    iota_b = bass.AP(tensor=iota_bf.tensor, offset=iota_bf.offset,
                                     ap=[iota_bf.ap[0], [0, t1 - t0],
                                         iota_bf.ap[1], [0, S]])
                    c4_b = bass.AP(tensor=c4s.tensor, offset=c4s.offset,
                                   ap=[c4s.ap[0], c4s.ap[1], [0, NC2], c4s.ap[2]])
                    eng.tensor_tensor(out=cmp[:, t0:t1], in0=iota_b, in1=c4_b,
                                      op=OP.is_equal)
                A1 = sbh.tile([128, NT, NC2, 4], BF, tag="A1")
                (nc.vector if pool_off else nc.gpsimd).tensor_tensor(
                    out=A1, in0=cmp[:, :, :, 0:4],
                    in1=cmp[:, :, :, 4:8], op=OP.add)
                A2 = sbh.tile([128, NT, NC2, 2], BF, tag="A2")
                nc.vector.tensor_tensor(out=A2, in0=A1[:, :, :, 0:2],
                                        in1=A1[:, :, :, 2:4], op=OP.add)
                nc.vector.tensor_tensor(out=TI[:, :, 64:128], in0=A2[:, :, :, 0],
                                        in1=A2[:, :, :, 1], op=OP.add)
                rl_b = bass.AP(tensor=rl.tensor, offset=rl.offset,
                               ap=[rl.ap[0], rl.ap[1], [0, NC2]])
                nc.vector.tensor_tensor(out=TI[:, :, 0:64], in0=TI[:, :, 64:128],
                                        in1=rl_b, op=OP.mult)
                psAB = psA.tile([128, CH], F32, tag="pA")
                for t in range(NT):
                    nc.tensor.transpose(psAB[:, t * 128:(t + 1) * 128], TI[:, t], ident)
                # rows 0:64 = hspT, 64:128 = hT; bin0/phantoms die in the tables
                hd = sbh.tile([128, CH], R, tag="hd")
                nc.scalar.activation(hd[0:64, :], psAB[0:64, :], AF.Relu)
                nd = sbh.tile([128, CH], R, tag="nd")
                nc.scalar.activation(nd[0:64, :], psAB[64:128, :], AF.Relu)
                if act_dup:
                    nc.scalar.activation(hd[64:128, :], psAB[0:64, :], AF.Relu)
                    nc.scalar.activation(nd[64:128, :], psAB[64:128, :], AF.Relu)
                else:
                    dma(out=hd[64:128, :], in_=hd[0:64, :])
                    dma(out=nd[64:128, :], in_=nd[0:64, :])

                # per head-pair: den -> W = hsp/den -> S -> U = n*S -> handsum
                hs_ps = psC.tile([E + 1, CH], F32, tag="pC")
                for g in range(2):
                    den_p = psB.tile([128, CH], F32, tag="pB")
                    nc.tensor.matmul(den_p, E2tabr[:, g * 128:(g + 1) * 128],
                                     nd[0:64, :], start=True, stop=True)
                    rden = sbh.tile([128, CH], F32, tag="rden")
                    nc.vector.reciprocal(rden, den_p)
                    Wg = sbh.tile([128, CH], R, tag="Wg")
                    nc.vector.tensor_tensor(out=Wg, in0=rden,
                                            in1=hd.bitcast(F32), op=OP.mult)
                    S_p = psB.tile([128, CH], F32, tag="pB")
                    nc.tensor.matmul(S_p, BDEf[g], Wg, start=True, stop=True)
                    Ug = sbh.tile([128, CH], R, tag="Ug")
                    nc.vector.tensor_tensor(out=Ug, in0=S_p,
                                            in1=nd.bitcast(F32), op=OP.mult)
                    nc.tensor.matmul(hs_ps, BDVP[g], Ug, start=(g == 0), stop=(g == 1))

                # hs97 = [handsum ; g2] feeds c1 as ONE matmul (m1cat)
                hs97 = sb.tile([128, CH], R, tag="hs97")
                (nc.vector if pool_off else nc.gpsimd).memset(
                    hs97.bitcast(F32)[64:96, :], 0.0)
                if dve_copies:
                    nc.vector.tensor_copy(hs97[0:E + 1, :], hs_ps)
                else:
                    nc.scalar.activation(hs97[0:E + 1, :], hs_ps, AF.Copy)

                # game-state MLP (g2s lands in hs97 rows 65:97)
                g1_ps = psC.tile([64, CH], F32, tag="pC")
                nc.tensor.matmul(g1_ps, g1wT, gsT, start=True, stop=True)
                g1s = sb.tile([64, CH], R, tag="g1s")
                nc.scalar.activation(g1s, g1_ps, AF.Relu, bias=g1b)
                g2_ps = psC.tile([32, CH], F32, tag="pC")
                nc.tensor.matmul(g2_ps, g2wT, g1s, start=True, stop=True)
                nc.scalar.activation(hs97[96:128, :], g2_ps, AF.Relu, bias=g2b)

                c1_ps = psC.tile([128, CH], F32, tag="pC")
                nc.tensor.matmul(c1_ps, m1cat, hs97, start=True, stop=True)
                ctx1 = sb.tile([128, CH], R, tag="ctx1")
                nc.scalar.activation(ctx1, c1_ps, AF.Relu, bias=c1b)
                c2_ps = psC.tile([128, CH], F32, tag="pC")
                nc.tensor.matmul(c2_ps, c2wT, ctx1, start=True, stop=True)
                ctx2 = sb.tile([128, CH], R, tag="ctx2")
                nc.scalar.activation(ctx2, c2_ps, AF.Relu, bias=c2b)

                # s1 ctx part duplicated -> P1d [128, CH] bf16 (h1 runs 2x on DVE)
                p1_ps = psC.tile([128, CH], F32, tag="pC")
                nc.tensor.matmul(p1_ps, s1wAd, ctx2, start=True, stop=True)
                P1d = sb.tile([128, CH], BF, tag="P1d")
                if dve_copies:
                    nc.vector.tensor_copy(P1d, p1_ps)
                else:
                    nc.scalar.activation(P1d, p1_ps, AF.Copy)

                # actions: h1 = relu(P1 + Ptab), h2 = relu(BDs2 @ h1),
                # s3 via matmul(lhsT=h2-slice, rhs=BDw3) -> [b, a] layout
                h2s = []
                for jj in range(5):
                    h2 = sbh2.tile([128, CH], BF, tag="h2", name=f"h2_{jj}")
                    hp = psD.tile([128, CH], F32, tag="pD", name=f"h2p{jj}")
                    for half in range(2):
                        j = jj * 2 + half
                        h1 = sbh.tile([128, CH], BF, tag="h1")
                        k = j % 10
                        if k < gps_h1:
                            nc.gpsimd.tensor_scalar(h1, P1d, Ptab2[:, j:j + 1], 0.0,
                                                    OP.add, OP.max)
                        elif k < gps_h1 + act_h1:
                            nc.scalar.activation(h1, P1d, AF.Relu, bias=Ptab2[:, j:j + 1])
                        else:
                            nc.vector.tensor_scalar(h1, P1d, Ptab2[:, j:j + 1], 0.0,
                                                    OP.add, OP.max)
                        nc.tensor.matmul(hp, BDs2L if half == 0 else BDs2R, h1,
                                         start=(half == 0), stop=(half == 1))
                    if jj < act_h2:
                        nc.scalar.activation(h2, hp, AF.Relu, bias=s2b4)
                    else:
                        nc.vector.tensor_scalar(h2, hp, s2b4, 0.0, OP.add, OP.max)
                    h2s.append(h2)
                for t in range(NT):
                    so_t = psD.tile([128, A], F32, tag="pD", name=f"so{t}")
                    for jj in range(5):
                        nc.tensor.matmul(so_t[:, jj * 4:(jj + 1) * 4],
                                         h2s[jj][:, t * 128:(t + 1) * 128], BDw3,
                                         start=True, stop=True)
                    nc.vector.tensor_scalar_add(out_sb[:, cix * NT + t, :], so_t,
                                                s3b_bc)
                if nva < A:
                    nc.vector.memset(out_sb[:, cix * NT:(cix + 1) * NT, nva:A], -1e8)

            # One For_i iteration = one FULL rep: all chunks python-unrolled
            # with compile-time DRAM offsets (no dynamic DMA, full cross-chunk
            # pipelining); reps only changes the loop bound, so the NEFF size
            # — and with it the per-call client overhead — is constant.
            ET = mybir.EngineType
            with tc.For_i(0, reps, 1,
                          hint_engines=((ET.PE, ET.DVE, ET.Activation, ET.Pool,
                                         ET.SP) if hints else ())):
                rep_prologue()
                for cix in range(nchunks):
                    body(cix)
                dma(out=out_d[:, :, :], in_=out_sb)

    legalize_multiwait(nc)
    return nc


_NC_CACHE = {}


def _get_nc(b_core, nva):
    key = (b_core, nva)
    if key not in _NC_CACHE:
        _NC_CACHE[key] = build_nc(b_core=b_core, nva=nva)
    return _NC_CACHE[key]


def pack_core(hc, gs, hs):
    """Host-side repack of one core's logical inputs into the kernel's
    DMA-friendly DRAM layouts (big contiguous rows, loaded to SBUF once)."""
    bc = hc.shape[0]
    tb = bc // 128
    return dict(
        hand_cards=np.ascontiguousarray(
            np.asarray(hc).reshape(tb, 128, S).transpose(1, 0, 2)).astype(np.int32),
        game_state=np.ascontiguousarray(np.asarray(gs).T).astype(np.float32),
        hand_size=np.ascontiguousarray(
            np.asarray(hs).reshape(tb, 128).T).astype(np.int32),
    )


def unpack_out(out, bc):
    """[128, TB, A] core output -> logical [bc, A]."""
    tb = bc // 128
    return np.ascontiguousarray(
        np.asarray(out).reshape(128, tb, A).transpose(1, 0, 2).reshape(bc, A))


def make_in_maps(inputs, b_core=BC, ncores=NCORES):
    shard_keys = ("hand_cards", "game_state", "hand_size")
    rep = {k: np.ascontiguousarray(v) for k, v in inputs.items()
           if k not in shard_keys and k != "num_valid_actions"}
    in_maps = []
    for c in range(ncores):
        m = dict(rep)
        m.update(pack_core(inputs["hand_cards"][c * b_core:(c + 1) * b_core],
                           inputs["game_state"][c * b_core:(c + 1) * b_core],
                           inputs["hand_size"][c * b_core:(c + 1) * b_core]))
        in_maps.append(m)
    return in_maps


def kernel(**inputs):
    from concourse.bass_utils import run_bass_kernel_spmd

    nva = int(inputs["num_valid_actions"])
    nc = _get_nc(BC, nva)
    in_maps = make_in_maps(inputs)
    res = run_bass_kernel_spmd(nc, in_maps, list(range(NCORES)))
    out = np.concatenate([unpack_out(res.results[c]["out"], BC)
                          for c in range(NCORES)], axis=0)
    return out.astype(np.float32)

